# revision 26
# baseline (speedup 1.0000x reference)
"""Trainium2 Bass kernel for nn_ExtensibleAttention (sparse deformable-style attention).

Math (reference.py):
  q = query@Wq.T + pos@Wp.T ; k = key@Wk.T + pos@Wp.T ; v = value@Wv.T      (all + biases)
  sp = reference_points (+ offsets from Woff — zeros in this problem)
  k_s, v_s = bilinear_sample(k_map, sp), bilinear_sample(v_map, sp)          (zeros padding)
  a = (q·k_s)/sqrt(D) per head ; w = softmax over the 8 heads
  out = (w * v_s) @ Wout.T + bout

Sharding: 8 cores = 4 batches x 2 sequence halves. Each core builds the full
9216-entry k||v feature map for its batch (replicated across the pair), writes
it to DRAM in bf16, then bilinear-gathers 4 corner rows per query token with
dma_gather (int16 indices computed on device), and does the token-major
attention math with DVE/ACT, PE for matmuls/transposes.

Fast path requires Woff/boff == 0 (true for this problem's setup_inputs); a
numpy fallback handles the general case exactly.
"""

import os
import sys

import numpy as np

if "/opt/trn_rl_repo" not in sys.path:
    sys.path.insert(0, "/opt/trn_rl_repo")

import concourse.bacc as bacc
import concourse.mybir as mybir
import concourse.tile as tile
from concourse import library_config
from concourse.bass_utils import run_bass_kernel_spmd
from concourse.mybir import ActivationFunctionType as AFT
from concourse.mybir import AluOpType as ALU
import bass_rust

F32 = mybir.dt.float32
BF16 = mybir.dt.bfloat16
I16 = mybir.dt.int16
NP_BF16 = mybir.dt.np(BF16)

P = 128
C = 256
CH = 2            # channel chunks of 128
H = 8
D = 32
HF = WF = 96
LMAP = HF * WF    # 9216
LLOC = LMAP // 2  # 4608 query tokens per core
NT_MAP = LMAP // P   # 72
NT_Q = LLOC // P     # 36
STRIP = 9            # tiles per load strip
KVROW = 2 * C        # 512 bf16 elements per map row (k || v)
N_CORES = 8
INV_SQRT_D = 1.0 / np.sqrt(np.float32(D))
OFF = 256.0          # floor-trick offset (coords shifted positive)


def _floorize(nc, pool, src_ap, n, tag):
    """dst = floor(src) elementwise for src > 0, exact under any f32->int
    conversion rounding mode (trunc / nearest / floor / ceil)."""
    ti = pool.tile([P, n], I16, tag=f"{tag}_i")
    nc.vector.tensor_copy(ti[:], src_ap)
    tf = pool.tile([P, n], F32, tag=f"{tag}_f")
    nc.vector.tensor_copy(tf[:], ti[:])
    gt = pool.tile([P, n], F32, tag=f"{tag}_g")
    nc.vector.tensor_tensor(out=gt[:], in0=tf[:], in1=src_ap, op=ALU.is_gt)
    dst = pool.tile([P, n], F32, tag=f"{tag}_o")
    nc.vector.tensor_tensor(out=dst[:], in0=tf[:], in1=gt[:], op=ALU.subtract)
    return dst


def build_program():
    nc = bacc.Bacc("TRN2", target_bir_lowering=False, debug=False,
                   num_devices=N_CORES)

    keyT = nc.dram_tensor("keyT", [C, LMAP], BF16, kind="ExternalInput")
    valueT = nc.dram_tensor("valueT", [C, LMAP], BF16, kind="ExternalInput")
    posT = nc.dram_tensor("posT", [C, LMAP], BF16, kind="ExternalInput")
    queryT = nc.dram_tensor("queryT", [C, LLOC], BF16, kind="ExternalInput")
    posqT = nc.dram_tensor("posqT", [C, LLOC], BF16, kind="ExternalInput")
    ref = nc.dram_tensor("ref", [LLOC, 2], F32, kind="ExternalInput")
    wkT = nc.dram_tensor("wkT", [C, C], BF16, kind="ExternalInput")
    wvT = nc.dram_tensor("wvT", [C, C], BF16, kind="ExternalInput")
    wpT = nc.dram_tensor("wpT", [C, C], BF16, kind="ExternalInput")
    wqT = nc.dram_tensor("wqT", [C, C], BF16, kind="ExternalInput")
    woT = nc.dram_tensor("woT", [C, C], BF16, kind="ExternalInput")
    repmat = nc.dram_tensor("repmat", [16, P], F32, kind="ExternalInput")
    identity = nc.dram_tensor("identity", [P, P], BF16, kind="ExternalInput")
    out = nc.dram_tensor("out", [LLOC, C], F32, kind="ExternalOutput")

    kv_map = nc.dram_tensor("kv_map", [LMAP + 1, KVROW], BF16, kind="Internal")

    with tile.TileContext(nc) as tc:
        with (
            tc.tile_pool(name="const", bufs=1) as const,
            tc.tile_pool(name="pre", bufs=1) as pre,
            tc.tile_pool(name="kstrip", bufs=3) as kstrip_p,
            tc.tile_pool(name="vstrip", bufs=3) as vstrip_p,
            tc.tile_pool(name="pstrip", bufs=3) as pstrip_p,
            tc.tile_pool(name="qstrip", bufs=2) as qstrip_p,
            tc.tile_pool(name="pqstrip", bufs=2) as pqstrip_p,
            tc.tile_pool(name="kv", bufs=4) as kv_p,
            tc.tile_pool(name="gat", bufs=6) as gat_p,
            tc.tile_pool(name="att", bufs=6) as att_p,
            tc.tile_pool(name="small", bufs=8) as small_p,
            tc.tile_pool(name="ost", bufs=3) as ost_p,
            tc.tile_pool(name="psA", bufs=3, space="PSUM") as psA,
            tc.tile_pool(name="psB", bufs=2, space="PSUM") as psB,
            tc.tile_pool(name="psC", bufs=3, space="PSUM") as psC,
        ):
            # ---- constants ----
            def load_w(t):
                sb = const.tile([P, CH, C], BF16, tag=f"w_{t.name}")
                nc.sync.dma_start(sb[:], t.ap().rearrange("(ch p) n -> p ch n", p=P))
                return sb
            wk_sb, wv_sb, wp_sb, wq_sb, wo_sb = (
                load_w(t) for t in (wkT, wvT, wpT, wqT, woT))
            ident_sb = const.tile([P, P], BF16, tag="ident")
            nc.sync.dma_start(ident_sb[:], identity.ap())
            repmat_sb = const.tile([16, P], F32, tag="repmat")
            nc.sync.dma_start(repmat_sb[:], repmat.ap())
            zero_sb = const.tile([P, KVROW], BF16, tag="zero")
            nc.vector.memset(zero_sb[:], 0.0)
            nc.sync.dma_start(kv_map.ap()[LMAP:LMAP + 1, :], zero_sb[0:1, :])

            # ---- ref point loads ----
            # token-major: partition p <- token j*128+p
            rt = const.tile([P, NT_Q, 2], F32, tag="rt")
            nc.sync.dma_start(rt[:], ref.ap().rearrange("(j p) c -> p j c", p=P))
            # wrapped (for gather indices): partition p<16 <- token j*128+a*16+p
            rw16 = const.tile([16, NT_Q, 8, 2], F32, tag="rw16")
            nc.sync.dma_start(
                rw16[:],
                ref.ap().rearrange("(j a p) c -> p j a c", p=16, a=8))
            rw16f = rw16[:].rearrange("p j a c -> p (j a c)")
            # replicate to all 128 partitions via matmul with repmat
            rw = const.tile([P, NT_Q, 8, 2], F32, tag="rw")
            rwf = rw[:].rearrange("p j a c -> p (j a c)")
            for lo, hi in ((0, 512), (512, 576)):
                rwps = psC.tile([P, 512], F32, space="PSUM", tag="Q")
                nc.tensor.matmul(rwps[:, 0:hi - lo], repmat_sb[:],
                                 rw16f[:, lo:hi], start=True, stop=True)
                nc.scalar.activation(rwf[:, lo:hi], rwps[:, 0:hi - lo], AFT.Copy)

            # ---- gather indices (wrapped layout) ----
            NW = NT_Q * 8  # 288
            idx_all = const.tile([P, NT_Q, 2, 8], I16, tag="idx_all")
            xs_w = pre.tile([P, NW], F32, tag="xs_w")
            nc.vector.tensor_scalar(xs_w[:], rw[:, :, :, 0], float(WF), OFF - 0.5,
                                    ALU.mult, ALU.add)
            ys_w = pre.tile([P, NW], F32, tag="ys_w")
            nc.vector.tensor_scalar(ys_w[:], rw[:, :, :, 1], float(HF), OFF - 0.5,
                                    ALU.mult, ALU.add)
            fx_w = _floorize(nc, pre, xs_w[:], NW, "fxw")
            fy_w = _floorize(nc, pre, ys_w[:], NW, "fyw")

            def clampc(f_ap, delta, tag):
                # clamp(floor+delta - OFF, 0, 95)
                t = pre.tile([P, NW], F32, tag=tag)
                nc.vector.tensor_scalar(t[:], f_ap, delta - OFF, 0.0,
                                        ALU.add, ALU.max)
                nc.vector.tensor_scalar(t[:], t[:], float(WF - 1), None, ALU.min)
                return t
            # x start of each gathered pair: max(x0, 0) (<=95 already)
            xp = pre.tile([P, NW], F32, tag="xp")
            nc.vector.tensor_scalar(xp[:], fx_w[:], -OFF, 0.0, ALU.add, ALU.max)
            y0c = clampc(fy_w[:], 0.0, "y0c")
            y1c = clampc(fy_w[:], 1.0, "y1c")
            for pidx, yc in enumerate((y0c, y1c)):
                idxf = pre.tile([P, NW], F32, tag="idxf")
                nc.vector.scalar_tensor_tensor(
                    out=idxf[:], in0=yc[:], scalar=float(WF), in1=xp[:],
                    op0=ALU.mult, op1=ALU.add)
                nc.vector.tensor_copy(
                    out=idx_all[:, :, pidx, :],
                    in_=idxf[:].rearrange("p (j a) -> p j a", a=8))

            # ---- bilinear weights (token-major) ----
            w_all = const.tile([P, NT_Q, 4], F32, tag="w_all")
            xs_t = pre.tile([P, NT_Q], F32, tag="xs_t")
            nc.vector.tensor_scalar(xs_t[:], rt[:, :, 0], float(WF), OFF - 0.5,
                                    ALU.mult, ALU.add)
            ys_t = pre.tile([P, NT_Q], F32, tag="ys_t")
            nc.vector.tensor_scalar(ys_t[:], rt[:, :, 1], float(HF), OFF - 0.5,
                                    ALU.mult, ALU.add)
            fx_t = _floorize(nc, pre, xs_t[:], NT_Q, "fxt")
            fy_t = _floorize(nc, pre, ys_t[:], NT_Q, "fyt")

            def masks(f_ap, tag):
                m0 = pre.tile([P, NT_Q], F32, tag=f"{tag}_m0")
                nc.vector.tensor_scalar(m0[:], f_ap, OFF - 0.5, None, ALU.is_ge)
                m1 = pre.tile([P, NT_Q], F32, tag=f"{tag}_m1")
                nc.vector.tensor_scalar(m1[:], f_ap, OFF + float(WF - 2) + 0.5,
                                        None, ALU.is_le)
                return m0, m1

            def frac(s_ap, f_ap, tag):
                w = pre.tile([P, NT_Q], F32, tag=f"{tag}_w")
                nc.vector.tensor_tensor(out=w[:], in0=s_ap, in1=f_ap, op=ALU.subtract)
                return w
            wx = frac(xs_t[:], fx_t[:], "wx")
            wy = frac(ys_t[:], fy_t[:], "wy")
            mx0, mx1 = masks(fx_t[:], "mx")
            my0, my1 = masks(fy_t[:], "my")
            # slot weights within a pair; x0=-1 case shifts x1's weight into
            # the lo slot (the pair was gathered starting at x=0)
            # ux_lo = mx0 + wx - 2*wx*mx0 ; ux_hi = wx*mx1*mx0
            t_wm = pre.tile([P, NT_Q], F32, tag="t_wm")
            nc.vector.tensor_tensor(out=t_wm[:], in0=wx[:], in1=mx0[:], op=ALU.mult)
            t_s = pre.tile([P, NT_Q], F32, tag="t_s")
            nc.vector.tensor_tensor(out=t_s[:], in0=mx0[:], in1=wx[:], op=ALU.add)
            ux0 = pre.tile([P, NT_Q], F32, tag="ux0")
            nc.vector.scalar_tensor_tensor(out=ux0[:], in0=t_wm[:], scalar=-2.0,
                                           in1=t_s[:], op0=ALU.mult, op1=ALU.add)
            t_mm = pre.tile([P, NT_Q], F32, tag="t_mm")
            nc.vector.tensor_tensor(out=t_mm[:], in0=mx1[:], in1=mx0[:], op=ALU.mult)
            ux1 = pre.tile([P, NT_Q], F32, tag="ux1")
            nc.vector.tensor_tensor(out=ux1[:], in0=wx[:], in1=t_mm[:], op=ALU.mult)
            # y corner weights (pairs are separate gathers; plain masking)
            uy0 = pre.tile([P, NT_Q], F32, tag="uy0")
            om = pre.tile([P, NT_Q], F32, tag="om")
            nc.vector.tensor_scalar(om[:], wy[:], -1.0, 1.0, ALU.mult, ALU.add)
            nc.vector.tensor_tensor(out=uy0[:], in0=om[:], in1=my0[:], op=ALU.mult)
            uy1 = pre.tile([P, NT_Q], F32, tag="uy1")
            nc.vector.tensor_tensor(out=uy1[:], in0=wy[:], in1=my1[:], op=ALU.mult)
            corners_u = [(uy0, ux0), (uy0, ux1), (uy1, ux0), (uy1, ux1)]
            for cidx, (uy, ux) in enumerate(corners_u):
                nc.vector.tensor_tensor(out=w_all[:, :, cidx], in0=uy[:],
                                        in1=ux[:], op=ALU.mult)

            nc.gpsimd.load_library(library_config.mlp)

            # ---- map phase: kv_map[s] = [k(s) || v(s)] in bf16 ----
            LS = STRIP * P  # strip length in tokens
            WB = 4          # map tiles batched per kv_map write DMA
            kvt = None
            for s in range(NT_MAP // STRIP):
                k_st = kstrip_p.tile([P, CH, LS], BF16)
                v_st = vstrip_p.tile([P, CH, LS], BF16)
                p_st = pstrip_p.tile([P, CH, LS], BF16)
                # split loads across the two HWDGE rings (SP + ACT)
                nc.sync.dma_start(
                    k_st[:], keyT.ap().rearrange("(ch p) l -> p ch l", p=P)
                    [:, :, s * LS:(s + 1) * LS])
                nc.sync.dma_start(
                    p_st[:], posT.ap().rearrange("(ch p) l -> p ch l", p=P)
                    [:, :, s * LS:(s + 1) * LS])
                nc.sync.dma_start(
                    v_st[:], valueT.ap().rearrange("(ch p) l -> p ch l", p=P)
                    [:, :, s * LS:(s + 1) * LS])
                for jj in range(STRIP):
                    sl = slice(jj * P, (jj + 1) * P)
                    kp = psA.tile([P, C], F32, space="PSUM", tag="A")
                    nc.tensor.matmul(kp[:], k_st[:, 0, sl], wk_sb[:, 0, :],
                                     start=True, stop=False)
                    nc.tensor.matmul(kp[:], k_st[:, 1, sl], wk_sb[:, 1, :],
                                     start=False, stop=False)
                    nc.tensor.matmul(kp[:], p_st[:, 0, sl], wp_sb[:, 0, :],
                                     start=False, stop=False)
                    nc.tensor.matmul(kp[:], p_st[:, 1, sl], wp_sb[:, 1, :],
                                     start=False, stop=True)
                    vp = psB.tile([P, C], F32, space="PSUM", tag="B")
                    nc.tensor.matmul(vp[:], v_st[:, 0, sl], wv_sb[:, 0, :],
                                     start=True, stop=False)
                    nc.tensor.matmul(vp[:], v_st[:, 1, sl], wv_sb[:, 1, :],
                                     start=False, stop=True)
                    j = s * STRIP + jj
                    if j % WB == 0:
                        kvt = kv_p.tile([P, WB, KVROW], BF16, name="kvt")
                    nc.scalar.activation(kvt[:, j % WB, 0:C], kp[:], AFT.Copy)
                    nc.scalar.activation(kvt[:, j % WB, C:KVROW], vp[:], AFT.Copy)
                    if j % WB == WB - 1:
                        j0 = j - (WB - 1)
                        nc.scalar.dma_start(
                            kv_map.ap()[j0 * P:(j0 + WB) * P, :]
                            .rearrange("(jj p) e -> p jj e", p=P),
                            kvt[:])

            # ---- q projection interleaved with attention ----
            ot = None
            q_st = pq_st = None
            for j in range(NT_Q):
                if j % STRIP == 0:
                    sidx = j // STRIP
                    q_st = qstrip_p.tile([P, CH, LS], BF16, name="q_st")
                    pq_st = pqstrip_p.tile([P, CH, LS], BF16, name="pq_st")
                    for st, t in ((q_st, queryT), (pq_st, posqT)):
                        nc.sync.dma_start(
                            st[:],
                            t.ap().rearrange("(ch p) l -> p ch l", p=P)
                            [:, :, sidx * LS:(sidx + 1) * LS])
                sl = slice((j % STRIP) * P, (j % STRIP + 1) * P)
                qp = psC.tile([P, C], F32, space="PSUM", tag="Q", name="qp")
                nc.tensor.matmul(qp[:], q_st[:, 0, sl], wq_sb[:, 0, :],
                                 start=True, stop=False)
                nc.tensor.matmul(qp[:], q_st[:, 1, sl], wq_sb[:, 1, :],
                                 start=False, stop=False)
                nc.tensor.matmul(qp[:], pq_st[:, 0, sl], wp_sb[:, 0, :],
                                 start=False, stop=False)
                nc.tensor.matmul(qp[:], pq_st[:, 1, sl], wp_sb[:, 1, :],
                                 start=False, stop=True)
                g = gat_p.tile([P, 2, 2 * KVROW], BF16)
                kv_pairs = bass_rust.AP(tensor=kv_map.ap().tensor, offset=0,
                                        ap=[[KVROW, LMAP], [1, 2 * KVROW]])
                nc.gpsimd.dma_gather(
                    out_ap=g[:],
                    in_ap=kv_pairs,
                    idxs_ap=idx_all[:, j, :, :].rearrange("p c a -> p (c a)"),
                    num_idxs=2 * P,
                    num_idxs_reg=2 * P,
                    elem_size=2 * KVROW,
                    elem_step=KVROW,
                )

                # corner c -> (pair, slot): 0:(y0,x0) 1:(y0,x1) 2:(y1,x0) 3:(y1,x1)
                corner_slc = ((0, 0), (0, 1), (1, 0), (1, 1))

                def gslc(cidx, base):
                    pair, slot = corner_slc[cidx]
                    return g[:, pair, slot * KVROW + base:slot * KVROW + base + C]

                def combine(base, tag):
                    # two parallel 2-deep STT chains + one add (shorter
                    # cross-tile critical path than a 4-deep chain)
                    h0 = att_p.tile([P, C], BF16, tag=f"{tag}_h0")
                    nc.vector.scalar_tensor_tensor(
                        out=h0[:], in0=gslc(0, base),
                        scalar=w_all[:, j, 0:1], in1=zero_sb[:, 0:C],
                        op0=ALU.mult, op1=ALU.add)
                    nc.vector.scalar_tensor_tensor(
                        out=h0[:], in0=gslc(1, base),
                        scalar=w_all[:, j, 1:2], in1=h0[:],
                        op0=ALU.mult, op1=ALU.add)
                    h1 = att_p.tile([P, C], BF16, tag=f"{tag}_h1")
                    nc.vector.scalar_tensor_tensor(
                        out=h1[:], in0=gslc(2, base),
                        scalar=w_all[:, j, 2:3], in1=zero_sb[:, 0:C],
                        op0=ALU.mult, op1=ALU.add)
                    nc.vector.scalar_tensor_tensor(
                        out=h1[:], in0=gslc(3, base),
                        scalar=w_all[:, j, 3:4], in1=h1[:],
                        op0=ALU.mult, op1=ALU.add)
                    acc = att_p.tile([P, C], BF16, tag=tag)
                    nc.vector.tensor_tensor(out=acc[:], in0=h0[:], in1=h1[:],
                                            op=ALU.add)
                    return acc

                def combine_act(base, tag):
                    # ACT per-partition-scaled copies + DVE add tree
                    sc = [att_p.tile([P, C], BF16, tag=f"{tag}_s{i}",
                                     name=f"{tag}_s{i}") for i in range(4)]
                    for cidx in range(4):
                        nc.scalar.activation(sc[cidx][:], gslc(cidx, base),
                                             AFT.Copy,
                                             scale=w_all[:, j, cidx:cidx + 1])
                    t0 = att_p.tile([P, C], BF16, tag=f"{tag}_t0")
                    nc.vector.tensor_tensor(out=t0[:], in0=sc[0][:], in1=sc[1][:],
                                            op=ALU.add)
                    t1 = att_p.tile([P, C], BF16, tag=f"{tag}_t1")
                    nc.vector.tensor_tensor(out=t1[:], in0=sc[2][:], in1=sc[3][:],
                                            op=ALU.add)
                    acc = att_p.tile([P, C], BF16, tag=tag)
                    nc.vector.tensor_tensor(out=acc[:], in0=t0[:], in1=t1[:],
                                            op=ALU.add)
                    return acc
                ks = combine(0, "ks")
                tmp = att_p.tile([P, C], BF16, tag="tmp")
                nc.vector.tensor_tensor(out=tmp[:], in0=qp[:],
                                        in1=ks[:], op=ALU.mult)
                a = small_p.tile([P, H], F32, tag="a")
                nc.vector.reduce_sum(out=a[:],
                                     in_=tmp[:].rearrange("p (h d) -> p h d", d=D),
                                     axis=mybir.AxisListType.X)
                e = small_p.tile([P, H], F32, tag="e")
                ssum = small_p.tile([P, 1], F32, tag="ssum")
                nc.scalar.activation(e[:], a[:], AFT.Exp,
                                     scale=float(INV_SQRT_D),
                                     accum_out=ssum[:])
                rinv = small_p.tile([P, 1], F32, tag="rinv")
                nc.vector.reciprocal(rinv[:], ssum[:])

                vs = combine_act(C, "vs")
                outs = att_p.tile([P, C], BF16, tag="outs")
                nc.vector.scalar_tensor_tensor(
                    out=outs[:].rearrange("p (h d) -> p h d", d=D),
                    in0=vs[:].rearrange("p (h d) -> p h d", d=D),
                    scalar=rinv[:],
                    in1=e[:].to_broadcast([P, H, D]),
                    op0=ALU.mult, op1=ALU.mult)

                tp = psA.tile([P, C], BF16, space="PSUM", tag="A")
                nc.tensor.transpose(tp[:, 0:P], outs[:, 0:P], ident_sb[:])
                nc.tensor.transpose(tp[:, P:C], outs[:, P:C], ident_sb[:])
                oT = att_p.tile([P, C], BF16, tag="oT")
                nc.scalar.activation(oT[:], tp[:], AFT.Copy)
                fp = psB.tile([P, C], F32, space="PSUM", tag="B")
                nc.tensor.matmul(fp[:], oT[:, 0:P], wo_sb[:, 0, :],
                                 start=True, stop=False)
                nc.tensor.matmul(fp[:], oT[:, P:C], wo_sb[:, 1, :],
                                 start=False, stop=True)
                if j % 4 == 0:
                    ot = ost_p.tile([P, 4, C], F32, name="ot")
                nc.scalar.activation(ot[:, j % 4, :], fp[:], AFT.Copy)
                if j % 4 == 3:
                    j0 = j - 3
                    nc.sync.dma_start(
                        out.ap()[j0 * P:(j0 + 4) * P, :]
                        .rearrange("(jj p) e -> p jj e", p=P),
                        ot[:])

    nc.compile()
    return nc


_PROGRAM = None


def _get_program():
    global _PROGRAM
    if _PROGRAM is None:
        _PROGRAM = build_program()
    return _PROGRAM


def _reference_numpy(query, key, value, reference_points, pos_embed,
                     Wq, bq, Wk, bk, Wv, bv, Wp, bp, Woff, boff, Wout, bout,
                     h_feat, w_feat):
    """Exact numpy fallback (only used if Woff/boff nonzero)."""
    N, L, Cc = query.shape
    Hn = H
    Dn = Cc // Hn
    q = (query @ Wq.T + bq).reshape(N, L, Hn, Dn)
    k = (key @ Wk.T + bk).reshape(N, L, Hn, Dn)
    v = (value @ Wv.T + bv).reshape(N, L, Hn, Dn)
    pos = (pos_embed @ Wp.T + bp).reshape(N, L, Hn, Dn)
    q = q + pos
    k = k + pos
    offsets = (query @ Woff.T + boff).reshape(N, L, Hn, 2)
    sp = reference_points[:, :, None, :] + offsets
    k_map = k.reshape(N, h_feat, w_feat, Hn, Dn)
    v_map = v.reshape(N, h_feat, w_feat, Hn, Dn)

    def bil(feat, pts):
        x = pts[..., 0] * w_feat - 0.5
        y = pts[..., 1] * h_feat - 0.5
        x0 = np.floor(x).astype(np.int64)
        y0 = np.floor(y).astype(np.int64)
        wx = x - x0
        wy = y - y0
        res = 0.0
        for yi, xi, wgt in ((y0, x0, (1 - wy) * (1 - wx)),
                            (y0, x0 + 1, (1 - wy) * wx),
                            (y0 + 1, x0, wy * (1 - wx)),
                            (y0 + 1, x0 + 1, wy * wx)):
            valid = ((yi >= 0) & (yi < h_feat) & (xi >= 0) & (xi < w_feat))
            yc = np.clip(yi, 0, h_feat - 1)
            xc = np.clip(xi, 0, w_feat - 1)
            n_idx = np.arange(N)[:, None, None]
            h_idx = np.arange(Hn)[None, None, :]
            gathered = feat[n_idx, yc, xc, h_idx]
            res = res + gathered * (wgt * valid)[..., None]
        return res
    k_s = bil(k_map, sp)
    v_s = bil(v_map, sp)
    a = np.einsum('nlhd,nlhd->nlh', q, k_s) / np.sqrt(np.float32(Dn))
    a = a - a.max(axis=-1, keepdims=True)
    ex = np.exp(a)
    w = ex / ex.sum(axis=-1, keepdims=True)
    o = (w[..., None] * v_s).reshape(N, L, Cc)
    return (o @ Wout.T + bout).astype(np.float32)


def kernel(**inputs):
    query = np.asarray(inputs["query"], np.float32)
    key = np.asarray(inputs["key"], np.float32)
    value = np.asarray(inputs["value"], np.float32)
    ref_pts = np.asarray(inputs["reference_points"], np.float32)
    pos = np.asarray(inputs["pos_embed"], np.float32)
    Wq = np.asarray(inputs["Wq"], np.float32); bq = np.asarray(inputs["bq"], np.float32)
    Wk = np.asarray(inputs["Wk"], np.float32); bk = np.asarray(inputs["bk"], np.float32)
    Wv = np.asarray(inputs["Wv"], np.float32); bv = np.asarray(inputs["bv"], np.float32)
    Wp = np.asarray(inputs["Wp"], np.float32); bp = np.asarray(inputs["bp"], np.float32)
    Woff = np.asarray(inputs["Woff"], np.float32); boff = np.asarray(inputs["boff"], np.float32)
    Wout = np.asarray(inputs["Wout"], np.float32); bout = np.asarray(inputs["bout"], np.float32)
    h_feat = int(inputs["h_feat"]); w_feat = int(inputs["w_feat"])

    N, L, Cc = query.shape
    general = (np.any(Woff) or np.any(boff) or np.any(bq) or np.any(bk)
               or np.any(bv) or np.any(bp) or np.any(bout)
               or h_feat != HF or w_feat != WF or (N, L, Cc) != (4, LMAP, C))
    if general:
        return _reference_numpy(query, key, value, ref_pts, pos,
                                Wq, bq, Wk, bk, Wv, bv, Wp, bp, Woff, boff,
                                Wout, bout, h_feat, w_feat)

    nc = _get_program()

    wk = np.ascontiguousarray(Wk.T).astype(NP_BF16)
    wv = np.ascontiguousarray(Wv.T).astype(NP_BF16)
    wp = np.ascontiguousarray(Wp.T).astype(NP_BF16)
    wq = np.ascontiguousarray(Wq.T).astype(NP_BF16)
    wo = np.ascontiguousarray(Wout.T).astype(NP_BF16)
    repmat = np.zeros((16, P), np.float32)
    repmat[np.arange(P) % 16, np.arange(P)] = 1.0
    ident = np.eye(P, dtype=np.float32).astype(NP_BF16)

    keyT_n = [key[n].T.astype(NP_BF16) for n in range(N)]
    valueT_n = [value[n].T.astype(NP_BF16) for n in range(N)]
    posT_n = [pos[n].T.astype(NP_BF16) for n in range(N)]

    in_maps = []
    for c in range(N_CORES):
        n, half = c // 2, c % 2
        sl = slice(half * LLOC, (half + 1) * LLOC)
        in_maps.append({
            "keyT": keyT_n[n],
            "valueT": valueT_n[n],
            "posT": posT_n[n],
            "queryT": query[n, sl].T.astype(NP_BF16),
            "posqT": pos[n, sl].T.astype(NP_BF16),
            "ref": np.ascontiguousarray(ref_pts[n, sl]),
            "wkT": wk, "wvT": wv, "wpT": wp, "wqT": wq, "woT": wo,
            "repmat": repmat, "identity": ident,
        })

    res = run_bass_kernel_spmd(nc, in_maps, list(range(N_CORES)),
                               **_RUN_KWARGS)
    if _RESULT_HOOK is not None:
        _RESULT_HOOK(res)
    full = np.empty((N, LMAP, C), np.float32)
    for c in range(N_CORES):
        n, half = c // 2, c % 2
        full[n, half * LLOC:(half + 1) * LLOC] = res.results[c]["out"]
    return full


# test.py hooks (harmless defaults for standalone grading)
_RUN_KWARGS: dict = {}
_RESULT_HOOK = None


# revision 27
# speedup vs baseline: 1.1558x; 1.1558x over previous
"""Trainium2 Bass kernel for nn_ExtensibleAttention (sparse deformable-style attention).

Math (reference.py):
  q = query@Wq.T + pos@Wp.T ; k = key@Wk.T + pos@Wp.T ; v = value@Wv.T      (all + biases)
  sp = reference_points (+ offsets from Woff — zeros in this problem)
  k_s, v_s = bilinear_sample(k_map, sp), bilinear_sample(v_map, sp)          (zeros padding)
  a = (q·k_s)/sqrt(D) per head ; w = softmax over the 8 heads
  out = (w * v_s) @ Wout.T + bout

Sharding: 8 cores = 4 batches x 2 sequence halves. Each core builds the full
9216-entry k||v feature map for its batch (replicated across the pair), writes
it to DRAM in bf16, then bilinear-gathers 4 corner rows per query token with
dma_gather (int16 indices computed on device), and does the token-major
attention math with DVE/ACT, PE for matmuls/transposes.

Fast path requires Woff/boff == 0 (true for this problem's setup_inputs); a
numpy fallback handles the general case exactly.
"""

import os
import sys

import numpy as np

if "/opt/trn_rl_repo" not in sys.path:
    sys.path.insert(0, "/opt/trn_rl_repo")

import concourse.bacc as bacc
import concourse.mybir as mybir
import concourse.tile as tile
from concourse import library_config
from concourse.bass_utils import run_bass_kernel_spmd
from concourse.mybir import ActivationFunctionType as AFT
from concourse.mybir import AluOpType as ALU
import bass_rust

F32 = mybir.dt.float32
BF16 = mybir.dt.bfloat16
I16 = mybir.dt.int16
NP_BF16 = mybir.dt.np(BF16)

P = 128
C = 256
CH = 2            # channel chunks of 128
H = 8
D = 32
HF = WF = 96
LMAP = HF * WF    # 9216
LLOC = LMAP // 2  # 4608 query tokens per core
NT_MAP = LMAP // P   # 72
NT_Q = LLOC // P     # 36
STRIP = 9            # tiles per load strip
KVROW = 2 * C        # 512 bf16 elements per map row (k || v)
N_CORES = 8
INV_SQRT_D = 1.0 / np.sqrt(np.float32(D))
OFF = 256.0          # floor-trick offset (coords shifted positive)


def _floorize(nc, pool, src_ap, n, tag):
    """dst = floor(src) elementwise for src > 0, exact under any f32->int
    conversion rounding mode (trunc / nearest / floor / ceil)."""
    ti = pool.tile([P, n], I16, tag=f"{tag}_i")
    nc.vector.tensor_copy(ti[:], src_ap)
    tf = pool.tile([P, n], F32, tag=f"{tag}_f")
    nc.vector.tensor_copy(tf[:], ti[:])
    gt = pool.tile([P, n], F32, tag=f"{tag}_g")
    nc.vector.tensor_tensor(out=gt[:], in0=tf[:], in1=src_ap, op=ALU.is_gt)
    dst = pool.tile([P, n], F32, tag=f"{tag}_o")
    nc.vector.tensor_tensor(out=dst[:], in0=tf[:], in1=gt[:], op=ALU.subtract)
    return dst


def build_program():
    nc = bacc.Bacc("TRN2", target_bir_lowering=False, debug=False,
                   num_devices=N_CORES)

    keyT = nc.dram_tensor("keyT", [C, LMAP], BF16, kind="ExternalInput")
    valueT = nc.dram_tensor("valueT", [C, LMAP], BF16, kind="ExternalInput")
    posT = nc.dram_tensor("posT", [C, LMAP], BF16, kind="ExternalInput")
    queryT = nc.dram_tensor("queryT", [C, LLOC], BF16, kind="ExternalInput")
    posqT = nc.dram_tensor("posqT", [C, LLOC], BF16, kind="ExternalInput")
    ref = nc.dram_tensor("ref", [LLOC, 2], F32, kind="ExternalInput")
    wkT = nc.dram_tensor("wkT", [C, C], BF16, kind="ExternalInput")
    wvT = nc.dram_tensor("wvT", [C, C], BF16, kind="ExternalInput")
    wpT = nc.dram_tensor("wpT", [C, C], BF16, kind="ExternalInput")
    wqT = nc.dram_tensor("wqT", [C, C], BF16, kind="ExternalInput")
    woT = nc.dram_tensor("woT", [C, C], BF16, kind="ExternalInput")
    repmat = nc.dram_tensor("repmat", [16, P], F32, kind="ExternalInput")
    identity = nc.dram_tensor("identity", [P, P], BF16, kind="ExternalInput")
    out = nc.dram_tensor("out", [LLOC, C], F32, kind="ExternalOutput")

    kv_map = nc.dram_tensor("kv_map", [LMAP + 1, KVROW], BF16, kind="Internal")

    with tile.TileContext(nc) as tc:
        with (
            tc.tile_pool(name="const", bufs=1) as const,
            tc.tile_pool(name="pre", bufs=1) as pre,
            tc.tile_pool(name="kstrip", bufs=3) as kstrip_p,
            tc.tile_pool(name="vstrip", bufs=3) as vstrip_p,
            tc.tile_pool(name="pstrip", bufs=3) as pstrip_p,
            tc.tile_pool(name="qstrip", bufs=2) as qstrip_p,
            tc.tile_pool(name="pqstrip", bufs=2) as pqstrip_p,
            tc.tile_pool(name="kv", bufs=4) as kv_p,
            tc.tile_pool(name="gat", bufs=6) as gat_p,
            tc.tile_pool(name="att", bufs=6) as att_p,
            tc.tile_pool(name="small", bufs=8) as small_p,
            tc.tile_pool(name="ost", bufs=3) as ost_p,
            tc.tile_pool(name="psA", bufs=3, space="PSUM") as psA,
            tc.tile_pool(name="psB", bufs=2, space="PSUM") as psB,
            tc.tile_pool(name="psC", bufs=3, space="PSUM") as psC,
        ):
            # ---- constants ----
            def load_w(t):
                sb = const.tile([P, CH, C], BF16, tag=f"w_{t.name}")
                nc.sync.dma_start(sb[:], t.ap().rearrange("(ch p) n -> p ch n", p=P))
                return sb
            wk_sb, wv_sb, wp_sb, wq_sb, wo_sb = (
                load_w(t) for t in (wkT, wvT, wpT, wqT, woT))
            ident_sb = const.tile([P, P], BF16, tag="ident")
            nc.sync.dma_start(ident_sb[:], identity.ap())
            repmat_sb = const.tile([16, P], F32, tag="repmat")
            nc.sync.dma_start(repmat_sb[:], repmat.ap())
            zero_sb = const.tile([P, KVROW], BF16, tag="zero")
            nc.vector.memset(zero_sb[:], 0.0)
            nc.sync.dma_start(kv_map.ap()[LMAP:LMAP + 1, :], zero_sb[0:1, :])

            # ---- ref point loads ----
            # token-major: partition p <- token j*128+p
            rt = const.tile([P, NT_Q, 2], F32, tag="rt")
            nc.sync.dma_start(rt[:], ref.ap().rearrange("(j p) c -> p j c", p=P))
            # wrapped (for gather indices): partition p<16 <- token j*128+a*16+p
            rw16 = const.tile([16, NT_Q, 8, 2], F32, tag="rw16")
            nc.sync.dma_start(
                rw16[:],
                ref.ap().rearrange("(j a p) c -> p j a c", p=16, a=8))
            rw16f = rw16[:].rearrange("p j a c -> p (j a c)")
            # replicate to all 128 partitions via matmul with repmat
            rw = const.tile([P, NT_Q, 8, 2], F32, tag="rw")
            rwf = rw[:].rearrange("p j a c -> p (j a c)")
            for lo, hi in ((0, 512), (512, 576)):
                rwps = psC.tile([P, 512], F32, space="PSUM", tag="Q")
                nc.tensor.matmul(rwps[:, 0:hi - lo], repmat_sb[:],
                                 rw16f[:, lo:hi], start=True, stop=True)
                nc.scalar.activation(rwf[:, lo:hi], rwps[:, 0:hi - lo], AFT.Copy)

            # ---- gather indices (wrapped layout) ----
            NW = NT_Q * 8  # 288
            idx_all = const.tile([P, NT_Q, 2, 8], I16, tag="idx_all")
            xs_w = pre.tile([P, NW], F32, tag="xs_w")
            nc.vector.tensor_scalar(xs_w[:], rw[:, :, :, 0], float(WF), OFF - 0.5,
                                    ALU.mult, ALU.add)
            ys_w = pre.tile([P, NW], F32, tag="ys_w")
            nc.vector.tensor_scalar(ys_w[:], rw[:, :, :, 1], float(HF), OFF - 0.5,
                                    ALU.mult, ALU.add)
            fx_w = _floorize(nc, pre, xs_w[:], NW, "fxw")
            fy_w = _floorize(nc, pre, ys_w[:], NW, "fyw")

            def clampc(f_ap, delta, tag):
                # clamp(floor+delta - OFF, 0, 95)
                t = pre.tile([P, NW], F32, tag=tag)
                nc.vector.tensor_scalar(t[:], f_ap, delta - OFF, 0.0,
                                        ALU.add, ALU.max)
                nc.vector.tensor_scalar(t[:], t[:], float(WF - 1), None, ALU.min)
                return t
            # x start of each gathered pair: max(x0, 0) (<=95 already)
            xp = pre.tile([P, NW], F32, tag="xp")
            nc.vector.tensor_scalar(xp[:], fx_w[:], -OFF, 0.0, ALU.add, ALU.max)
            y0c = clampc(fy_w[:], 0.0, "y0c")
            y1c = clampc(fy_w[:], 1.0, "y1c")
            for pidx, yc in enumerate((y0c, y1c)):
                idxf = pre.tile([P, NW], F32, tag="idxf")
                nc.vector.scalar_tensor_tensor(
                    out=idxf[:], in0=yc[:], scalar=float(WF), in1=xp[:],
                    op0=ALU.mult, op1=ALU.add)
                nc.vector.tensor_copy(
                    out=idx_all[:, :, pidx, :],
                    in_=idxf[:].rearrange("p (j a) -> p j a", a=8))

            # ---- bilinear weights (token-major) ----
            w_all = const.tile([P, NT_Q, 4], F32, tag="w_all")
            xs_t = pre.tile([P, NT_Q], F32, tag="xs_t")
            nc.vector.tensor_scalar(xs_t[:], rt[:, :, 0], float(WF), OFF - 0.5,
                                    ALU.mult, ALU.add)
            ys_t = pre.tile([P, NT_Q], F32, tag="ys_t")
            nc.vector.tensor_scalar(ys_t[:], rt[:, :, 1], float(HF), OFF - 0.5,
                                    ALU.mult, ALU.add)
            fx_t = _floorize(nc, pre, xs_t[:], NT_Q, "fxt")
            fy_t = _floorize(nc, pre, ys_t[:], NT_Q, "fyt")

            def masks(f_ap, tag):
                m0 = pre.tile([P, NT_Q], F32, tag=f"{tag}_m0")
                nc.vector.tensor_scalar(m0[:], f_ap, OFF - 0.5, None, ALU.is_ge)
                m1 = pre.tile([P, NT_Q], F32, tag=f"{tag}_m1")
                nc.vector.tensor_scalar(m1[:], f_ap, OFF + float(WF - 2) + 0.5,
                                        None, ALU.is_le)
                return m0, m1

            def frac(s_ap, f_ap, tag):
                w = pre.tile([P, NT_Q], F32, tag=f"{tag}_w")
                nc.vector.tensor_tensor(out=w[:], in0=s_ap, in1=f_ap, op=ALU.subtract)
                return w
            wx = frac(xs_t[:], fx_t[:], "wx")
            wy = frac(ys_t[:], fy_t[:], "wy")
            mx0, mx1 = masks(fx_t[:], "mx")
            my0, my1 = masks(fy_t[:], "my")
            # slot weights within a pair; x0=-1 case shifts x1's weight into
            # the lo slot (the pair was gathered starting at x=0)
            # ux_lo = mx0 + wx - 2*wx*mx0 ; ux_hi = wx*mx1*mx0
            t_wm = pre.tile([P, NT_Q], F32, tag="t_wm")
            nc.vector.tensor_tensor(out=t_wm[:], in0=wx[:], in1=mx0[:], op=ALU.mult)
            t_s = pre.tile([P, NT_Q], F32, tag="t_s")
            nc.vector.tensor_tensor(out=t_s[:], in0=mx0[:], in1=wx[:], op=ALU.add)
            ux0 = pre.tile([P, NT_Q], F32, tag="ux0")
            nc.vector.scalar_tensor_tensor(out=ux0[:], in0=t_wm[:], scalar=-2.0,
                                           in1=t_s[:], op0=ALU.mult, op1=ALU.add)
            t_mm = pre.tile([P, NT_Q], F32, tag="t_mm")
            nc.vector.tensor_tensor(out=t_mm[:], in0=mx1[:], in1=mx0[:], op=ALU.mult)
            ux1 = pre.tile([P, NT_Q], F32, tag="ux1")
            nc.vector.tensor_tensor(out=ux1[:], in0=wx[:], in1=t_mm[:], op=ALU.mult)
            # y corner weights (pairs are separate gathers; plain masking)
            uy0 = pre.tile([P, NT_Q], F32, tag="uy0")
            om = pre.tile([P, NT_Q], F32, tag="om")
            nc.vector.tensor_scalar(om[:], wy[:], -1.0, 1.0, ALU.mult, ALU.add)
            nc.vector.tensor_tensor(out=uy0[:], in0=om[:], in1=my0[:], op=ALU.mult)
            uy1 = pre.tile([P, NT_Q], F32, tag="uy1")
            nc.vector.tensor_tensor(out=uy1[:], in0=wy[:], in1=my1[:], op=ALU.mult)
            corners_u = [(uy0, ux0), (uy0, ux1), (uy1, ux0), (uy1, ux1)]
            for cidx, (uy, ux) in enumerate(corners_u):
                nc.vector.tensor_tensor(out=w_all[:, :, cidx], in0=uy[:],
                                        in1=ux[:], op=ALU.mult)

            nc.gpsimd.load_library(library_config.mlp)

            # ---- map phase: kv_map[s] = [k(s) || v(s)] in bf16 ----
            LS = STRIP * P  # strip length in tokens
            WB = 4          # map tiles batched per kv_map write DMA
            kvt = None
            for s in range(NT_MAP // STRIP):
                k_st = kstrip_p.tile([P, CH, LS], BF16)
                v_st = vstrip_p.tile([P, CH, LS], BF16)
                p_st = pstrip_p.tile([P, CH, LS], BF16)
                # split loads across the two HWDGE rings (SP + ACT)
                nc.sync.dma_start(
                    k_st[:], keyT.ap().rearrange("(ch p) l -> p ch l", p=P)
                    [:, :, s * LS:(s + 1) * LS])
                nc.sync.dma_start(
                    p_st[:], posT.ap().rearrange("(ch p) l -> p ch l", p=P)
                    [:, :, s * LS:(s + 1) * LS])
                nc.sync.dma_start(
                    v_st[:], valueT.ap().rearrange("(ch p) l -> p ch l", p=P)
                    [:, :, s * LS:(s + 1) * LS])
                for jj in range(STRIP):
                    sl = slice(jj * P, (jj + 1) * P)
                    kp = psA.tile([P, C], F32, space="PSUM", tag="A")
                    nc.tensor.matmul(kp[:], k_st[:, 0, sl], wk_sb[:, 0, :],
                                     start=True, stop=False)
                    nc.tensor.matmul(kp[:], k_st[:, 1, sl], wk_sb[:, 1, :],
                                     start=False, stop=False)
                    nc.tensor.matmul(kp[:], p_st[:, 0, sl], wp_sb[:, 0, :],
                                     start=False, stop=False)
                    nc.tensor.matmul(kp[:], p_st[:, 1, sl], wp_sb[:, 1, :],
                                     start=False, stop=True)
                    vp = psB.tile([P, C], F32, space="PSUM", tag="B")
                    nc.tensor.matmul(vp[:], v_st[:, 0, sl], wv_sb[:, 0, :],
                                     start=True, stop=False)
                    nc.tensor.matmul(vp[:], v_st[:, 1, sl], wv_sb[:, 1, :],
                                     start=False, stop=True)
                    j = s * STRIP + jj
                    if j % WB == 0:
                        kvt = kv_p.tile([P, WB, KVROW], BF16, name="kvt")
                    nc.scalar.activation(kvt[:, j % WB, 0:C], kp[:], AFT.Copy)
                    nc.scalar.activation(kvt[:, j % WB, C:KVROW], vp[:], AFT.Copy)
                    if j % WB == WB - 1:
                        j0 = j - (WB - 1)
                        nc.scalar.dma_start(
                            kv_map.ap()[j0 * P:(j0 + WB) * P, :]
                            .rearrange("(jj p) e -> p jj e", p=P),
                            kvt[:])

            # ---- q projection interleaved with attention ----
            q_all = const.tile([P, NT_Q, C], BF16, tag="q_all")

            def q_strip(sidx):
                q_st = qstrip_p.tile([P, CH, LS], BF16, name="q_st")
                pq_st = pqstrip_p.tile([P, CH, LS], BF16, name="pq_st")
                for st, t in ((q_st, queryT), (pq_st, posqT)):
                    nc.sync.dma_start(
                        st[:],
                        t.ap().rearrange("(ch p) l -> p ch l", p=P)
                        [:, :, sidx * LS:(sidx + 1) * LS])
                for jj in range(STRIP):
                    sl = slice(jj * P, (jj + 1) * P)
                    qp = psC.tile([P, C], F32, space="PSUM", tag="Q", name="qp")
                    nc.tensor.matmul(qp[:], q_st[:, 0, sl], wq_sb[:, 0, :],
                                     start=True, stop=False)
                    nc.tensor.matmul(qp[:], q_st[:, 1, sl], wq_sb[:, 1, :],
                                     start=False, stop=False)
                    nc.tensor.matmul(qp[:], pq_st[:, 0, sl], wp_sb[:, 0, :],
                                     start=False, stop=False)
                    nc.tensor.matmul(qp[:], pq_st[:, 1, sl], wp_sb[:, 1, :],
                                     start=False, stop=True)
                    nc.scalar.activation(q_all[:, sidx * STRIP + jj, :], qp[:],
                                         AFT.Copy)

            ot = None
            for j in range(NT_Q):
                if j % STRIP == 0:
                    q_strip(j // STRIP)
                g = gat_p.tile([P, 2, 2 * KVROW], BF16)
                kv_pairs = bass_rust.AP(tensor=kv_map.ap().tensor, offset=0,
                                        ap=[[KVROW, LMAP], [1, 2 * KVROW]])
                nc.gpsimd.dma_gather(
                    out_ap=g[:],
                    in_ap=kv_pairs,
                    idxs_ap=idx_all[:, j, :, :].rearrange("p c a -> p (c a)"),
                    num_idxs=2 * P,
                    num_idxs_reg=2 * P,
                    elem_size=2 * KVROW,
                    elem_step=KVROW,
                )

                # corner c -> (pair, slot): 0:(y0,x0) 1:(y0,x1) 2:(y1,x0) 3:(y1,x1)
                corner_slc = ((0, 0), (0, 1), (1, 0), (1, 1))

                def gslc(cidx, base):
                    pair, slot = corner_slc[cidx]
                    return g[:, pair, slot * KVROW + base:slot * KVROW + base + C]

                def combine(base, tag):
                    # two parallel 2-deep STT chains + one add (shorter
                    # cross-tile critical path than a 4-deep chain)
                    h0 = att_p.tile([P, C], BF16, tag=f"{tag}_h0")
                    nc.vector.scalar_tensor_tensor(
                        out=h0[:], in0=gslc(0, base),
                        scalar=w_all[:, j, 0:1], in1=zero_sb[:, 0:C],
                        op0=ALU.mult, op1=ALU.add)
                    nc.vector.scalar_tensor_tensor(
                        out=h0[:], in0=gslc(1, base),
                        scalar=w_all[:, j, 1:2], in1=h0[:],
                        op0=ALU.mult, op1=ALU.add)
                    h1 = att_p.tile([P, C], BF16, tag=f"{tag}_h1")
                    nc.vector.scalar_tensor_tensor(
                        out=h1[:], in0=gslc(2, base),
                        scalar=w_all[:, j, 2:3], in1=zero_sb[:, 0:C],
                        op0=ALU.mult, op1=ALU.add)
                    nc.vector.scalar_tensor_tensor(
                        out=h1[:], in0=gslc(3, base),
                        scalar=w_all[:, j, 3:4], in1=h1[:],
                        op0=ALU.mult, op1=ALU.add)
                    acc = att_p.tile([P, C], BF16, tag=tag)
                    nc.vector.tensor_tensor(out=acc[:], in0=h0[:], in1=h1[:],
                                            op=ALU.add)
                    return acc

                def combine_act(base, tag):
                    # ACT per-partition-scaled copies + DVE add tree
                    sc = [att_p.tile([P, C], BF16, tag=f"{tag}_s{i}",
                                     name=f"{tag}_s{i}") for i in range(4)]
                    for cidx in range(4):
                        nc.scalar.activation(sc[cidx][:], gslc(cidx, base),
                                             AFT.Copy,
                                             scale=w_all[:, j, cidx:cidx + 1])
                    t0 = att_p.tile([P, C], BF16, tag=f"{tag}_t0")
                    nc.vector.tensor_tensor(out=t0[:], in0=sc[0][:], in1=sc[1][:],
                                            op=ALU.add)
                    t1 = att_p.tile([P, C], BF16, tag=f"{tag}_t1")
                    nc.vector.tensor_tensor(out=t1[:], in0=sc[2][:], in1=sc[3][:],
                                            op=ALU.add)
                    acc = att_p.tile([P, C], BF16, tag=tag)
                    nc.vector.tensor_tensor(out=acc[:], in0=t0[:], in1=t1[:],
                                            op=ALU.add)
                    return acc
                ks = combine(0, "ks")
                tmp = att_p.tile([P, C], BF16, tag="tmp")
                nc.vector.tensor_tensor(out=tmp[:], in0=q_all[:, j, :],
                                        in1=ks[:], op=ALU.mult)
                a = small_p.tile([P, H], F32, tag="a")
                nc.vector.reduce_sum(out=a[:],
                                     in_=tmp[:].rearrange("p (h d) -> p h d", d=D),
                                     axis=mybir.AxisListType.X)
                e = small_p.tile([P, H], F32, tag="e")
                ssum = small_p.tile([P, 1], F32, tag="ssum")
                nc.scalar.activation(e[:], a[:], AFT.Exp,
                                     scale=float(INV_SQRT_D),
                                     accum_out=ssum[:])
                rinv = small_p.tile([P, 1], F32, tag="rinv")
                nc.vector.reciprocal(rinv[:], ssum[:])

                vs = combine_act(C, "vs")
                outs = att_p.tile([P, C], BF16, tag="outs")
                nc.vector.scalar_tensor_tensor(
                    out=outs[:].rearrange("p (h d) -> p h d", d=D),
                    in0=vs[:].rearrange("p (h d) -> p h d", d=D),
                    scalar=rinv[:],
                    in1=e[:].to_broadcast([P, H, D]),
                    op0=ALU.mult, op1=ALU.mult)

                tp = psA.tile([P, C], BF16, space="PSUM", tag="A")
                nc.tensor.transpose(tp[:, 0:P], outs[:, 0:P], ident_sb[:])
                nc.tensor.transpose(tp[:, P:C], outs[:, P:C], ident_sb[:])
                oT = att_p.tile([P, C], BF16, tag="oT")
                nc.scalar.activation(oT[:], tp[:], AFT.Copy)
                fp = psB.tile([P, C], F32, space="PSUM", tag="B")
                nc.tensor.matmul(fp[:], oT[:, 0:P], wo_sb[:, 0, :],
                                 start=True, stop=False)
                nc.tensor.matmul(fp[:], oT[:, P:C], wo_sb[:, 1, :],
                                 start=False, stop=True)
                if j % 4 == 0:
                    ot = ost_p.tile([P, 4, C], F32, name="ot")
                nc.scalar.activation(ot[:, j % 4, :], fp[:], AFT.Copy)
                if j % 4 == 3:
                    j0 = j - 3
                    nc.sync.dma_start(
                        out.ap()[j0 * P:(j0 + 4) * P, :]
                        .rearrange("(jj p) e -> p jj e", p=P),
                        ot[:])

    nc.compile()
    return nc


_PROGRAM = None


def _get_program():
    global _PROGRAM
    if _PROGRAM is None:
        _PROGRAM = build_program()
    return _PROGRAM


def _reference_numpy(query, key, value, reference_points, pos_embed,
                     Wq, bq, Wk, bk, Wv, bv, Wp, bp, Woff, boff, Wout, bout,
                     h_feat, w_feat):
    """Exact numpy fallback (only used if Woff/boff nonzero)."""
    N, L, Cc = query.shape
    Hn = H
    Dn = Cc // Hn
    q = (query @ Wq.T + bq).reshape(N, L, Hn, Dn)
    k = (key @ Wk.T + bk).reshape(N, L, Hn, Dn)
    v = (value @ Wv.T + bv).reshape(N, L, Hn, Dn)
    pos = (pos_embed @ Wp.T + bp).reshape(N, L, Hn, Dn)
    q = q + pos
    k = k + pos
    offsets = (query @ Woff.T + boff).reshape(N, L, Hn, 2)
    sp = reference_points[:, :, None, :] + offsets
    k_map = k.reshape(N, h_feat, w_feat, Hn, Dn)
    v_map = v.reshape(N, h_feat, w_feat, Hn, Dn)

    def bil(feat, pts):
        x = pts[..., 0] * w_feat - 0.5
        y = pts[..., 1] * h_feat - 0.5
        x0 = np.floor(x).astype(np.int64)
        y0 = np.floor(y).astype(np.int64)
        wx = x - x0
        wy = y - y0
        res = 0.0
        for yi, xi, wgt in ((y0, x0, (1 - wy) * (1 - wx)),
                            (y0, x0 + 1, (1 - wy) * wx),
                            (y0 + 1, x0, wy * (1 - wx)),
                            (y0 + 1, x0 + 1, wy * wx)):
            valid = ((yi >= 0) & (yi < h_feat) & (xi >= 0) & (xi < w_feat))
            yc = np.clip(yi, 0, h_feat - 1)
            xc = np.clip(xi, 0, w_feat - 1)
            n_idx = np.arange(N)[:, None, None]
            h_idx = np.arange(Hn)[None, None, :]
            gathered = feat[n_idx, yc, xc, h_idx]
            res = res + gathered * (wgt * valid)[..., None]
        return res
    k_s = bil(k_map, sp)
    v_s = bil(v_map, sp)
    a = np.einsum('nlhd,nlhd->nlh', q, k_s) / np.sqrt(np.float32(Dn))
    a = a - a.max(axis=-1, keepdims=True)
    ex = np.exp(a)
    w = ex / ex.sum(axis=-1, keepdims=True)
    o = (w[..., None] * v_s).reshape(N, L, Cc)
    return (o @ Wout.T + bout).astype(np.float32)


def kernel(**inputs):
    query = np.asarray(inputs["query"], np.float32)
    key = np.asarray(inputs["key"], np.float32)
    value = np.asarray(inputs["value"], np.float32)
    ref_pts = np.asarray(inputs["reference_points"], np.float32)
    pos = np.asarray(inputs["pos_embed"], np.float32)
    Wq = np.asarray(inputs["Wq"], np.float32); bq = np.asarray(inputs["bq"], np.float32)
    Wk = np.asarray(inputs["Wk"], np.float32); bk = np.asarray(inputs["bk"], np.float32)
    Wv = np.asarray(inputs["Wv"], np.float32); bv = np.asarray(inputs["bv"], np.float32)
    Wp = np.asarray(inputs["Wp"], np.float32); bp = np.asarray(inputs["bp"], np.float32)
    Woff = np.asarray(inputs["Woff"], np.float32); boff = np.asarray(inputs["boff"], np.float32)
    Wout = np.asarray(inputs["Wout"], np.float32); bout = np.asarray(inputs["bout"], np.float32)
    h_feat = int(inputs["h_feat"]); w_feat = int(inputs["w_feat"])

    N, L, Cc = query.shape
    general = (np.any(Woff) or np.any(boff) or np.any(bq) or np.any(bk)
               or np.any(bv) or np.any(bp) or np.any(bout)
               or h_feat != HF or w_feat != WF or (N, L, Cc) != (4, LMAP, C))
    if general:
        return _reference_numpy(query, key, value, ref_pts, pos,
                                Wq, bq, Wk, bk, Wv, bv, Wp, bp, Woff, boff,
                                Wout, bout, h_feat, w_feat)

    nc = _get_program()

    wk = np.ascontiguousarray(Wk.T).astype(NP_BF16)
    wv = np.ascontiguousarray(Wv.T).astype(NP_BF16)
    wp = np.ascontiguousarray(Wp.T).astype(NP_BF16)
    wq = np.ascontiguousarray(Wq.T).astype(NP_BF16)
    wo = np.ascontiguousarray(Wout.T).astype(NP_BF16)
    repmat = np.zeros((16, P), np.float32)
    repmat[np.arange(P) % 16, np.arange(P)] = 1.0
    ident = np.eye(P, dtype=np.float32).astype(NP_BF16)

    keyT_n = [key[n].T.astype(NP_BF16) for n in range(N)]
    valueT_n = [value[n].T.astype(NP_BF16) for n in range(N)]
    posT_n = [pos[n].T.astype(NP_BF16) for n in range(N)]

    in_maps = []
    for c in range(N_CORES):
        n, half = c // 2, c % 2
        sl = slice(half * LLOC, (half + 1) * LLOC)
        in_maps.append({
            "keyT": keyT_n[n],
            "valueT": valueT_n[n],
            "posT": posT_n[n],
            "queryT": query[n, sl].T.astype(NP_BF16),
            "posqT": pos[n, sl].T.astype(NP_BF16),
            "ref": np.ascontiguousarray(ref_pts[n, sl]),
            "wkT": wk, "wvT": wv, "wpT": wp, "wqT": wq, "woT": wo,
            "repmat": repmat, "identity": ident,
        })

    res = run_bass_kernel_spmd(nc, in_maps, list(range(N_CORES)),
                               **_RUN_KWARGS)
    if _RESULT_HOOK is not None:
        _RESULT_HOOK(res)
    full = np.empty((N, LMAP, C), np.float32)
    for c in range(N_CORES):
        n, half = c // 2, c % 2
        full[n, half * LLOC:(half + 1) * LLOC] = res.results[c]["out"]
    return full


# test.py hooks (harmless defaults for standalone grading)
_RUN_KWARGS: dict = {}
_RESULT_HOOK = None
